# revision 1
# baseline (speedup 1.0000x reference)
"""Trainium2 Bass kernel for factorized space-time attention.

Computation (per batch b of 8, one NeuronCore each):
  qkv = x @ w_qkv.T                      (3136, 2304)
  heads 0-5:  spatial attention over 196 patches within each of 16 frames
  heads 6-11: temporal attention over groups of 16 consecutive tokens
              (raw-reshape semantics of the reference: groups of 16
               consecutive n within each (b, head) slice)
  out = concat(head outputs) @ w_proj.T + b_proj

Strategy: data-parallel over batch (8 cores). All activations kept
feature-major ([d, n]) on chip so every matmul contraction runs over the
partition dim with no on-device transposes; x / weights are pre-transposed
host-side. V is additionally produced in token-major (natural) layout
directly by flipping the projection matmul orientation, chunked two ways:
per-frame [128+68] rows for spatial heads, uniform 112-row windows
(= 7 temporal groups) for temporal heads. Temporal attention is computed
on 112x112 score windows with a block-diagonal mask (7 x 16x16).
Softmax skips the max-subtraction (scores are ~N(0,1); exp is safe in
fp32) and normalizes via a ones-matmul partition-broadcast of 1/rowsum.
"""

import sys

# concourse normally comes from the axon site tree (sitecustomize); the
# append is a fallback so a bare environment still finds it.
if "/opt/trn_rl_repo" not in sys.path:
    sys.path.append("/opt/trn_rl_repo")

import numpy as np

import concourse.bass as bass  # noqa: F401  (engine namespaces live on nc)
import concourse.mybir as mybir
import concourse.tile as tile
from concourse import bacc
from concourse.bass_utils import run_bass_kernel_spmd

F32 = mybir.dt.float32
BF16 = mybir.dt.bfloat16
AF = mybir.ActivationFunctionType

# problem dims (hardcoded per contract)
B = 8
F = 16
P = 196
D = 768
NH = 12
HD = 64
N = F * P  # 3136
E3 = 3 * D  # 2304
SB = 784  # superblock = lcm(196, 16) tokens
NSB = N // SB  # 4
FPSB = SB // P  # 4 frames per superblock
WPSB = SB // 112  # 7 temporal windows per superblock
SCALE = HD ** -0.5

# compute dtype for matmul inputs ("f32" safest, "bf16" 2x faster on PE)
COMPUTE = "f32"

_CACHE = {}


def _build(compute: str, reps: int = 1):
    """Build + bass-compile the per-core kernel. Returns the Bacc object.

    compute: "f32" | "f32r" | "bf16" — dtype of matmul inputs. "f32r" keeps
    all data fp32 but runs the three projection matmul groups in the PE's
    faster reduced-precision fp32 mode via operand bitcasts.
    reps: device-side repetition count (for timing; wraps the body in For_i).
    """
    cdt = BF16 if compute == "bf16" else F32
    F32R = mybir.dt.float32r

    def mmcast(ap):
        return ap.bitcast(F32R) if compute == "f32r" else ap

    # bf16 tiles are half-size; spend the freed SBUF on double-buffering the
    # big per-superblock tiles so consecutive superblocks overlap fully.
    wb = 2 if compute == "bf16" else 1

    nc = bacc.Bacc("TRN2", target_bir_lowering=False, debug=False, num_devices=B)

    xt_d = nc.dram_tensor("xt", (D, N), cdt, kind="ExternalInput")
    wqkv_d = nc.dram_tensor("wqkvT", (D, E3), cdt, kind="ExternalInput")
    wproj_d = nc.dram_tensor("wprojT", (D, D), cdt, kind="ExternalInput")
    bias_d = nc.dram_tensor("bias", (D, 1), F32, kind="ExternalInput")
    mask_d = nc.dram_tensor("mask", (112, 112), cdt, kind="ExternalInput")
    out_d = nc.dram_tensor("outT", (D, N), F32, kind="ExternalOutput")

    with tile.TileContext(nc) as tc:
        with (
            tc.tile_pool(name="const", bufs=1) as cpool,
            tc.tile_pool(name="work", bufs=1) as wpool,
            tc.tile_pool(name="small", bufs=4) as spool,
            tc.tile_pool(name="psum", bufs=2, space="PSUM") as ppool,
        ):
            # ---- constants -------------------------------------------------
            wq = []
            for dc in range(6):
                t = cpool.tile([128, E3], cdt, tag=f"wq{dc}", name=f"wq{dc}")
                nc.sync.dma_start(t[:], wqkv_d.ap()[128 * dc : 128 * (dc + 1), :])
                wq.append(t)
            wp = []
            for dc in range(6):
                t = cpool.tile([128, D], cdt, tag=f"wp{dc}", name=f"wp{dc}")
                nc.sync.dma_start(t[:], wproj_d.ap()[128 * dc : 128 * (dc + 1), :])
                wp.append(t)
            bias_t = cpool.tile([128, 6], F32, tag="bias", name="bias_t")
            nc.sync.dma_start(
                bias_t[:], bias_d.ap().rearrange("(e p) one -> p (e one)", p=128)
            )
            mask2_t = cpool.tile([112, 224], cdt, tag="mask", name="mask2_t")
            nc.sync.dma_start(mask2_t[:, 0:112], mask_d.ap())
            nc.sync.dma_start(mask2_t[:, 112:224], mask_d.ap())
            zeros_col = cpool.tile([128, 1], F32, tag="zeros_c", name="zeros_col")
            nc.gpsimd.memset(zeros_col[:], 0.0)
            # row 64 of ones (matching the psum row the softmax sums land on)
            # is the stationary operand of the 1/sum partition-broadcast matmul
            ones64 = cpool.tile([65, 64], F32, tag="ones64", name="ones64")
            nc.gpsimd.memset(ones64[:], 1.0)

            import contextlib

            rep_ctx = tc.For_i(0, reps, 1) if reps > 1 else contextlib.nullcontext()
            with rep_ctx:
              for s in range(NSB):
                so = SB * s  # superblock token offset

                # ---- load x^T superblock ----------------------------------
                xts = []
                for dc in range(6):
                    t = wpool.tile([128, SB], cdt, tag=f"xts{dc}", bufs=wb, name=f"xts{dc}_{s}")
                    nc.sync.dma_start(
                        t[:], xt_d.ap()[128 * dc : 128 * (dc + 1), so : so + SB]
                    )
                    xts.append(t)

                # ---- QKV projection: Q,K regions, feature-major -----------
                # qkvt[t] rows = features 128t..128t+127 of [Q(768) | K(768)]
                qkvt = []
                for ti in range(12):
                    qt = wpool.tile([128, SB], cdt, tag=f"qkvt{ti}", bufs=wb, name=f"qkvt{ti}_{s}")
                    for j in range(2):
                        ps = ppool.tile([128, 392], F32, tag="mm", bufs=2, name=f"ps_qk{s}_{ti}_{j}")
                        for dc in range(6):
                            nc.tensor.matmul(
                                ps[:],
                                mmcast(wq[dc][:, 128 * ti : 128 * (ti + 1)]),
                                mmcast(xts[dc][:, 392 * j : 392 * (j + 1)]),
                                start=(dc == 0),
                                stop=(dc == 5),
                            )
                        nc.scalar.copy(qt[:, 392 * j : 392 * (j + 1)], ps[:])
                    qkvt.append(qt)

                # ---- V projection, token-major (natural) ------------------
                # layout per tile: 6 heads x [64 V-cols | ones-col] = 390 cols;
                # the ones column makes the AV matmul (M=65) emit the softmax
                # denominator as output row 64 for free.
                def v_proj(msz, tok0, wcol0, vtag, vname, psname):
                    vt_ = wpool.tile([msz, 390], cdt, tag=vtag, bufs=wb, name=vname)
                    ps = ppool.tile([msz, 384], F32, tag="mm", bufs=2, name=psname)
                    for dc in range(6):
                        nc.tensor.matmul(
                            ps[:],
                            mmcast(xts[dc][:, tok0 : tok0 + msz]),
                            mmcast(wq[dc][:, wcol0 : wcol0 + 384]),
                            start=(dc == 0),
                            stop=(dc == 5),
                        )
                    nc.scalar.copy(
                        vt_.rearrange("p (h c) -> p h c", c=65)[:, :, 0:64],
                        ps.rearrange("p (h c) -> p h c", c=64),
                    )
                    nc.gpsimd.memset(
                        vt_.rearrange("p (h c) -> p h c", c=65)[:, :, 64:65], 1.0
                    )
                    return vt_

                # spatial V: per-frame chunks of [128, 68] rows; cols = heads 0-5
                vs = []
                for f in range(FPSB):
                    for ci, (m0, msz) in enumerate(((0, 128), (128, 68))):
                        vs.append(
                            v_proj(msz, 196 * f + m0, 1536, f"vs{f}_{ci}",
                                   f"vs{f}_{ci}_{s}", f"ps_vs{s}_{f}_{ci}")
                        )
                # temporal V: uniform 112-token windows; cols = heads 6-11
                vt = []
                for w in range(WPSB):
                    vt.append(
                        v_proj(112, 112 * w, 1920, f"vt{w}",
                               f"vt{w}_{s}", f"ps_vt{s}_{w}")
                    )

                # ---- attention output, feature-major ----------------------
                attnT = [
                    wpool.tile([128, SB], cdt, tag=f"attnT{i}", bufs=wb,
                               name=f"attnT{i}_{s}")
                    for i in range(6)
                ]

                # ---- spatial attention (heads 0-5, per frame) --------------
                # one psum tile per accumulation group (HW requires a start/
                # stop group to own its bank); pairs share the 1/sum
                # reciprocal+broadcast stage.
                for f in range(FPSB):
                    fo = 196 * f
                    for hp in range(3):
                        ps_avs = []
                        for hi in range(2):
                            h = 2 * hp + hi
                            pb = 64 * hi
                            qtile = qkvt[h // 2]
                            ktile = qkvt[6 + h // 2]
                            es = []
                            for ci, (m0, msz) in enumerate(((0, 128), (128, 68))):
                                ps_st = ppool.tile(
                                    [msz, 196], F32, tag="st", bufs=3,
                                    name=f"ps_st{s}_{f}_{h}_{ci}",
                                )
                                nc.tensor.matmul(
                                    ps_st[:],
                                    ktile[pb : pb + 64, fo + m0 : fo + m0 + msz],
                                    qtile[pb : pb + 64, fo : fo + 196],
                                    start=True,
                                    stop=True,
                                )
                                e = spool.tile(
                                    [msz, 196], cdt, tag="e", bufs=6,
                                    name=f"e{s}_{f}_{h}_{ci}",
                                )
                                nc.scalar.activation(
                                    e[:], ps_st[:], AF.Exp,
                                    bias=zeros_col[:msz, :], scale=SCALE,
                                )
                                es.append(e)
                            # O^T numerator rows 0-63, softmax denom row 64
                            ps_av = ppool.tile(
                                [65, 196], F32, tag="av", bufs=2,
                                name=f"ps_sav{s}_{f}_{h}",
                            )
                            for ci in range(2):
                                nc.tensor.matmul(
                                    ps_av[:],
                                    vs[2 * f + ci][:, 65 * h : 65 * h + 65],
                                    es[ci][:],
                                    start=(ci == 0),
                                    stop=(ci == 1),
                                )
                            ps_avs.append(ps_av)
                        r = spool.tile([65, 392], F32, tag="r", name=f"r{s}_{f}_{hp}")
                        for hi in range(2):
                            nc.vector.reciprocal(
                                r[64:65, 196 * hi : 196 * hi + 196],
                                ps_avs[hi][64:65, :],
                            )
                        ps_b = ppool.tile(
                            [64, 392], F32, tag="mm", bufs=2, name=f"ps_b{s}_{f}_{hp}"
                        )
                        nc.tensor.matmul(
                            ps_b[:], ones64[64:65, :], r[64:65, :],
                            start=True, stop=True,
                        )
                        rb = spool.tile([64, 392], F32, tag="rb", name=f"rb{s}_{f}_{hp}")
                        nc.scalar.copy(rb[:], ps_b[:])
                        for hi in range(2):
                            h = 2 * hp + hi
                            cs = slice(196 * hi, 196 * hi + 196)
                            # even heads write attnT rows 0-63 directly, odd
                            # heads via tmp + SBUF->SBUF DMA (partition shift)
                            if hi == 0:
                                nc.vector.tensor_mul(
                                    attnT[h // 2][0:64, fo : fo + 196],
                                    ps_avs[hi][0:64, :], rb[:, cs],
                                )
                            else:
                                tmp = spool.tile(
                                    [64, 196], cdt, tag="tmp", name=f"tm{s}_{f}_{h}"
                                )
                                nc.vector.tensor_mul(
                                    tmp[:], ps_avs[hi][0:64, :], rb[:, cs]
                                )
                                nc.sync.dma_start(
                                    attnT[h // 2][64:128, fo : fo + 196], tmp[:]
                                )

                # ---- temporal attention (heads 6-11, per 112-window) -------
                for w in range(WPSB):
                    wo = 112 * w
                    for hp in range(3):
                        ps_avs = []
                        for hi in range(2):
                            h = 6 + 2 * hp + hi  # global head 6..11
                            pb = 64 * hi
                            ps_st = ppool.tile(
                                [112, 112], F32, tag="st", bufs=3,
                                name=f"ps_tst{s}_{w}_{h}",
                            )
                            nc.tensor.matmul(
                                ps_st[:],
                                qkvt[6 + h // 2][pb : pb + 64, wo : wo + 112],
                                qkvt[h // 2][pb : pb + 64, wo : wo + 112],
                                start=True,
                                stop=True,
                            )
                            e = spool.tile(
                                [112, 112], cdt, tag="e", bufs=6,
                                name=f"et{s}_{w}_{h}",
                            )
                            nc.scalar.activation(
                                e[:], ps_st[:], AF.Exp,
                                bias=zeros_col[:112, :], scale=SCALE,
                            )
                            em = spool.tile(
                                [112, 112], cdt, tag="e", bufs=6,
                                name=f"em{s}_{w}_{h}",
                            )
                            nc.vector.tensor_mul(em[:], e[:], mask2_t[:, 0:112])
                            ps_av = ppool.tile(
                                [65, 112], F32, tag="av", bufs=2,
                                name=f"ps_tav{s}_{w}_{h}",
                            )
                            nc.tensor.matmul(
                                ps_av[:],
                                vt[w][:, 65 * (h - 6) : 65 * (h - 6) + 65],
                                em[:],
                                start=True,
                                stop=True,
                            )
                            ps_avs.append(ps_av)
                        r = spool.tile([65, 224], F32, tag="r", name=f"rt{s}_{w}_{hp}")
                        for hi in range(2):
                            nc.vector.reciprocal(
                                r[64:65, 112 * hi : 112 * hi + 112],
                                ps_avs[hi][64:65, :],
                            )
                        ps_b = ppool.tile(
                            [64, 224], F32, tag="mm", bufs=2, name=f"ps_tb{s}_{w}_{hp}"
                        )
                        nc.tensor.matmul(
                            ps_b[:], ones64[64:65, :], r[64:65, :],
                            start=True, stop=True,
                        )
                        rb = spool.tile([64, 224], F32, tag="rb", name=f"rbt{s}_{w}_{hp}")
                        nc.scalar.copy(rb[:], ps_b[:])
                        for hi in range(2):
                            h = 6 + 2 * hp + hi
                            cs = slice(112 * hi, 112 * hi + 112)
                            at = attnT[3 + (h - 6) // 2]
                            if hi == 0:
                                nc.vector.tensor_mul(
                                    at[0:64, wo : wo + 112], ps_avs[hi][0:64, :],
                                    rb[:, cs],
                                )
                            else:
                                tmp = spool.tile(
                                    [64, 112], cdt, tag="tmp", name=f"tmt{s}_{w}_{h}"
                                )
                                nc.vector.tensor_mul(
                                    tmp[:], ps_avs[hi][0:64, :], rb[:, cs]
                                )
                                nc.sync.dma_start(
                                    at[64:128, wo : wo + 112], tmp[:]
                                )

                # ---- output projection + bias ------------------------------
                for ec in range(6):
                    for j in range(2):
                        ps = ppool.tile([128, 392], F32, tag="mm", bufs=2, name=f"ps_o{s}_{ec}_{j}")
                        for dc in range(6):
                            nc.tensor.matmul(
                                ps[:],
                                mmcast(wp[dc][:, 128 * ec : 128 * (ec + 1)]),
                                mmcast(attnT[dc][:, 392 * j : 392 * (j + 1)]),
                                start=(dc == 0),
                                stop=(dc == 5),
                            )
                        ot = spool.tile([128, 392], F32, tag="ot", name=f"ot{s}_{ec}_{j}")
                        nc.scalar.activation(
                            ot[:], ps[:], AF.Identity,
                            bias=bias_t[:, ec : ec + 1], scale=1.0,
                        )
                        nc.sync.dma_start(
                            out_d.ap()[
                                128 * ec : 128 * (ec + 1),
                                so + 392 * j : so + 392 * (j + 1),
                            ],
                            ot[:],
                        )

    nc.compile()
    return nc


def _get_nc(compute: str):
    if compute not in _CACHE:
        _CACHE[compute] = _build(compute)
    return _CACHE[compute]


def _np_dtype(compute: str):
    if compute == "f32":
        return np.float32
    import ml_dtypes

    return ml_dtypes.bfloat16


def kernel(x, w_qkv, w_proj, b_proj):
    nc = _get_nc(COMPUTE)
    dt = _np_dtype(COMPUTE)

    x = np.asarray(x, dtype=np.float32).reshape(B, N, D)
    xT = np.ascontiguousarray(x.transpose(0, 2, 1)).astype(dt)  # (B, D, N)
    wqkvT = np.ascontiguousarray(np.asarray(w_qkv, np.float32).T).astype(dt)
    wprojT = np.ascontiguousarray(np.asarray(w_proj, np.float32).T).astype(dt)
    bias = np.asarray(b_proj, np.float32).reshape(D, 1)

    mask = np.zeros((112, 112), np.float32)
    for g in range(7):
        mask[16 * g : 16 * (g + 1), 16 * g : 16 * (g + 1)] = 1.0
    mask = mask.astype(dt)

    in_maps = [
        {"xt": xT[b], "wqkvT": wqkvT, "wprojT": wprojT, "bias": bias, "mask": mask}
        for b in range(B)
    ]
    res = run_bass_kernel_spmd(nc, in_maps, core_ids=list(range(B)))
    out = np.stack([r["outT"].T for r in res.results])  # (B, N, D)
    return np.ascontiguousarray(out.reshape(B, F, P, D)).astype(np.float32)


if __name__ == "__main__":
    rng = np.random.default_rng(0)
    x = rng.standard_normal((B, F, P, D), dtype=np.float32)
    w_qkv = rng.standard_normal((E3, D), dtype=np.float32) * D**-0.5
    w_proj = rng.standard_normal((D, D), dtype=np.float32) * D**-0.5
    b_proj = np.zeros(D, np.float32)
    out = kernel(x=x, w_qkv=w_qkv, w_proj=w_proj, b_proj=b_proj)
    print(out.shape, out.dtype)



# revision 42
# speedup vs baseline: 1.6615x; 1.6615x over previous
"""Trainium2 Bass kernel for factorized space-time attention.

Computation (per batch b of 8, one NeuronCore each):
  qkv = x @ w_qkv.T                      (3136, 2304)
  heads 0-5:  spatial attention over 196 patches within each of 16 frames
  heads 6-11: temporal attention over groups of 16 consecutive tokens
  out = concat(head outputs) @ w_proj.T + b_proj

Strategy: data-parallel over batch (8 cores). All activations kept
feature-major ([d, n]) on chip so every matmul contraction runs over the
partition dim with no on-device transposes; x / weights are pre-transposed
host-side. V is produced token-major directly by flipping the projection
matmul orientation, chunked per-frame [128+68] rows for spatial heads and
uniform 112-row windows (= 7 temporal groups) for temporal heads, each
with a trailing ones column so the AV matmul (M=65) emits the softmax
denominator as output row 64 for free. Temporal attention runs on 112x112
score windows with a block-diagonal mask (7 x 16x16). Softmax skips the
max-subtraction (scores are ~N(0,1); exp is safe in fp32) and normalizes
via a ones-matmul partition-broadcast of 1/rowsum.

v2 engine-balance structure:
  - per-superblock batched DMAs: 1 input load, 2 odd-head partition-shift
    copies (collected in tmp_s/tmp_t staging tiles), 1 output store
  - temporal AV pairs share one PSUM tile (serial single-matmul groups),
    halving reciprocal count
  - QKV psum->sbuf copies on DVE, block-mask muls on GpSimd, exp +
    V/rb/output copies on ACT
  - output stored bf16 and upcast host-side (halves out DMA + SBUF)
"""

import sys

if "/opt/trn_rl_repo" not in sys.path:
    sys.path.append("/opt/trn_rl_repo")

import numpy as np

import concourse.bass as bass  # noqa: F401
import concourse.mybir as mybir
import concourse.tile as tile
from concourse import bacc
from concourse.bass_utils import run_bass_kernel_spmd

F32 = mybir.dt.float32
BF16 = mybir.dt.bfloat16
AF = mybir.ActivationFunctionType

B = 8
F = 16
P = 196
D = 768
NH = 12
HD = 64
N = F * P  # 3136
E3 = 3 * D  # 2304
SB = 784  # superblock = lcm(196, 16) tokens
NSB = N // SB  # 4
FPSB = SB // P  # 4 frames per superblock
WPSB = SB // 112  # 7 temporal windows per superblock
SCALE = HD ** -0.5

COMPUTE = "bf16"

_CACHE = {}


def _build(compute: str, reps: int = 1, phase: str = "all"):
    """Build + bass-compile the per-core kernel. Returns the Bacc object.

    phase: "all" | "proj" (QKV+V only, debug) | "noattn" (skip attention).
    """
    import os

    phase = os.environ.get("KPHASE", phase)
    cdt = BF16 if compute == "bf16" else F32
    F32R = mybir.dt.float32r

    def mmcast(ap):
        return ap.bitcast(F32R) if compute == "f32r" else ap

    wb = 2

    nc = bacc.Bacc("TRN2", target_bir_lowering=False, debug=False, num_devices=B)

    xt_d = nc.dram_tensor("xt", (D, N), cdt, kind="ExternalInput")
    wqkv_d = nc.dram_tensor("wqkvT", (D, E3), cdt, kind="ExternalInput")
    wproj_d = nc.dram_tensor("wprojT", (D, D), cdt, kind="ExternalInput")
    bias_d = nc.dram_tensor("bias", (D, 1), F32, kind="ExternalInput")
    mask_d = nc.dram_tensor("mask", (112, 112), cdt, kind="ExternalInput")
    out_d = nc.dram_tensor("outT", (D, N), cdt, kind="ExternalOutput")

    with tile.TileContext(nc) as tc:
        with (
            tc.tile_pool(name="const", bufs=1) as cpool,
            tc.tile_pool(name="work", bufs=1) as wpool,
            tc.tile_pool(name="small", bufs=4) as spool,
            tc.tile_pool(name="psum", bufs=2, space="PSUM") as ppool,
        ):
            # ---- constants -------------------------------------------------
            wq = []
            for dc in range(6):
                t = cpool.tile([128, E3], cdt, tag=f"wq{dc}", name=f"wq{dc}")
                nc.sync.dma_start(t[:], wqkv_d.ap()[128 * dc : 128 * (dc + 1), :])
                wq.append(t)
            wp = []
            for dc in range(6):
                t = cpool.tile([128, D], cdt, tag=f"wp{dc}", name=f"wp{dc}")
                nc.sync.dma_start(t[:], wproj_d.ap()[128 * dc : 128 * (dc + 1), :])
                wp.append(t)
            bias_t = cpool.tile([128, 6], F32, tag="bias", name="bias_t")
            nc.sync.dma_start(
                bias_t[:], bias_d.ap().rearrange("(e p) one -> p (e one)", p=128)
            )
            mask4_t = cpool.tile([112, 448], cdt, tag="mask", name="mask4_t")
            for mi in range(4):
                nc.sync.dma_start(mask4_t[:, 112 * mi : 112 * (mi + 1)], mask_d.ap())

            import contextlib

            rep_ctx = tc.For_i(0, reps, 1) if reps > 1 else contextlib.nullcontext()
            kbody = int(os.environ.get("KBODY", "1"))
            with rep_ctx:
              NS = NSB * kbody
              sbst = {}

              def prefetch_x(s):
                  so = SB * (s % NSB)
                  stt = sbst[s] = {"so": so}
                  xts = wpool.tile([128, 6 * SB], cdt, tag="xts", bufs=2,
                                   name=f"xts_{s}")
                  nc.sync.dma_start(
                      xts[:].rearrange("p (c n) -> p c n", n=SB),
                      xt_d.ap()[:, so : so + SB].rearrange(
                          "(c p) n -> p c n", p=128),
                  )
                  stt["xts"] = xts
                  stt["qkvt"] = [None] * 12
                  stt["vs"] = [None] * 8
                  stt["vt"] = [None] * 7

              def proj_units(s):
                  stt = sbst[s]

                  def xsl(dc, t0, t1):
                      return stt["xts"][:, SB * dc + t0 : SB * dc + t1]

                  units = []

                  def qkv_unit(ti, j):
                      if j == 0:
                          stt["qkvt"][ti] = wpool.tile(
                              [128, SB], cdt, tag=f"qkvt{ti}", bufs=wb,
                              name=f"qkvt{ti}_{s}")
                      qt = stt["qkvt"][ti]
                      ps = ppool.tile([128, 392], F32, tag="mm", bufs=3,
                                      name=f"ps_qk{s}_{ti}_{j}")
                      for dc in range(6):
                          nc.tensor.matmul(
                              ps[:],
                              mmcast(wq[dc][:, 128 * ti : 128 * (ti + 1)]),
                              mmcast(xsl(dc, 392 * j, 392 * (j + 1))),
                              start=(dc == 0),
                              stop=(dc == 5),
                          )
                      nc.vector.tensor_copy(
                          qt[:, 392 * j : 392 * (j + 1)], ps[:])

                  def v_unit(kind, idx, msz, tok0, wcol0):
                      # per-head layout [V_h (64) | ones (64)]: the AV
                      # stationary [k, 128] emits the head output in psum
                      # rows 0-63 AND the softmax denominator broadcast
                      # across rows 64-127 in the same matmul.
                      vt_ = wpool.tile([msz, 768], cdt, tag=f"v{kind}{idx}",
                                       bufs=wb, name=f"v{kind}{idx}_{s}")
                      ps = ppool.tile([msz, 384], F32, tag="mm", bufs=3,
                                      name=f"ps_v{kind}{s}_{idx}")
                      for dc in range(6):
                          nc.tensor.matmul(
                              ps[:],
                              mmcast(xsl(dc, tok0, tok0 + msz)),
                              mmcast(wq[dc][:, wcol0 : wcol0 + 384]),
                              start=(dc == 0),
                              stop=(dc == 5),
                          )
                      nc.vector.tensor_copy(
                          vt_.rearrange("p (h c) -> p h c", c=128)[:, :, 0:64],
                          ps.rearrange("p (h c) -> p h c", c=64),
                      )
                      nc.gpsimd.memset(
                          vt_.rearrange("p (h c) -> p h c", c=128)[:, :, 64:128],
                          1.0,
                      )
                      stt[f"v{kind}"][idx] = vt_

                  for ti in range(12):
                      for j in range(2):
                          units.append(lambda ti=ti, j=j: qkv_unit(ti, j))
                  for f in range(FPSB):
                      for ci in range(2):
                          units.append(lambda f=f, ci=ci: v_unit(
                              "s", 2 * f + ci, 98, 196 * f + 98 * ci, 1536))
                  for w in range(WPSB):
                      units.append(lambda w=w: v_unit("t", w, 112, 112 * w, 1920))
                  return units

              def attn_emitters(s):
                  """A/B/C staged attention ticks for superblock s; each
                  tick emits stage A of step n, B of n-1, C of n-2 so every
                  PE instruction's cross-engine deps were issued a full
                  step earlier."""
                  stt = sbst[s]
                  qkvt, vs, vt = stt["qkvt"], stt["vs"], stt["vt"]
                  attnT = wpool.tile([128, 6 * SB], cdt, tag="attnT", bufs=wb,
                                     name=f"attnT_{s}")
                  tmp_s = wpool.tile([64, 3 * SB], cdt, tag="tmp_s", bufs=wb,
                                     name=f"tmp_s_{s}")
                  tmp_t = wpool.tile([64, 3 * SB], cdt, tag="tmp_t", bufs=wb,
                                     name=f"tmp_t_{s}")
                  stt["attnT"] = attnT

                  if phase == "noattn":
                      def memset_unit():
                          nc.gpsimd.memset(attnT[:], 0.0)
                      return [memset_unit]
                  if phase == "proj":
                      return []

                  def sp_A(f, hp):
                      fo = 196 * f
                      es = []
                      for hi in range(2):
                          h = 2 * hp + hi
                          pb = 64 * hi
                          qtile = qkvt[h // 2]
                          ktile = qkvt[6 + h // 2]
                          # both 98-token k-chunks share one PSUM bank: same
                          # row group -> serial matmuls -> safe (row-group-
                          # disjoint pairs' concurrent drains into a shared
                          # bank fault the PE)
                          ps_st = ppool.tile(
                              [98, 392], F32, tag="st", bufs=3,
                              name=f"ps_st{s}_{f}_{h}",
                          )
                          for ci in range(2):
                              nc.tensor.matmul(
                                  ps_st[0:98, 196 * ci : 196 * ci + 196],
                                  ktile[pb : pb + 64,
                                        fo + 98 * ci : fo + 98 * ci + 98],
                                  qtile[pb : pb + 64, fo : fo + 196],
                                  start=True,
                                  stop=True,
                              )
                          e = spool.tile(
                              [98, 392], cdt, tag="e", bufs=12,
                              name=f"e{s}_{f}_{h}",
                          )
                          nc.scalar.activation(e[:], ps_st[:], AF.Exp,
                                               scale=SCALE)
                          es.append(e)
                      return {"es": es}

                  def sp_B(f, hp, st):
                      # two hi accumulation groups share one PSUM bank:
                      # readiness is gated by the in-order ACT exp chain, so
                      # emitted order is program order and hi=1's whole-bank
                      # has_written clear cannot land inside hi=0's group.
                      ps_av = ppool.tile(
                          [128, 392], F32, tag="ab", bufs=2,
                          name=f"ps_sav{s}_{f}_{hp}",
                      )
                      for hi in range(2):
                          h = 2 * hp + hi
                          for ci in range(2):
                              nc.tensor.matmul(
                                  ps_av[:, 196 * hi : 196 * hi + 196],
                                  vs[2 * f + ci][:, 128 * h : 128 * h + 128],
                                  st["es"][hi][0:98, 196 * ci : 196 * ci + 196],
                                  start=(ci == 0),
                                  stop=(ci == 1),
                              )
                      # 1/d = exp(-ln d) on ACT: Ln and Exp share the
                      # natural_log_exp table set -> no table switching
                      lt = spool.tile([64, 392], F32, tag="ln", bufs=4,
                                      name=f"l{s}_{f}_{hp}")
                      nc.scalar.activation(lt[:], ps_av[64:128, :], AF.Ln)
                      rb = spool.tile([64, 392], F32, tag="rb", bufs=4,
                                      name=f"rb{s}_{f}_{hp}")
                      nc.scalar.activation(rb[:], lt[:], AF.Exp, scale=-1.0)
                      st["av"] = ps_av
                      st["rb"] = rb

                  def sp_C(f, hp, st):
                      fo = 196 * f
                      nc.vector.tensor_mul(
                          attnT[0:64, SB * hp + fo : SB * hp + fo + 196],
                          st["av"][0:64, 0:196], st["rb"][:, 0:196],
                      )
                      nc.vector.tensor_mul(
                          tmp_s[:, SB * hp + fo : SB * hp + fo + 196],
                          st["av"][0:64, 196:392], st["rb"][:, 196:392],
                      )

                  def tm_A(cw, hp):
                      wid = 112 * len(cw)
                      ems = []
                      for hi in range(2):
                          h = 6 + 2 * hp + hi
                          pb = 64 * hi
                          ps_st = ppool.tile(
                              [112, wid], F32, tag="st", bufs=3,
                              name=f"ps_tst{s}_{cw[0]}_{h}",
                          )
                          for k, w in enumerate(cw):
                              nc.tensor.matmul(
                                  ps_st[:, 112 * k : 112 * k + 112],
                                  qkvt[6 + h // 2][pb : pb + 64,
                                                   112 * w : 112 * w + 112],
                                  qkvt[h // 2][pb : pb + 64,
                                               112 * w : 112 * w + 112],
                                  start=True,
                                  stop=True,
                              )
                          e = spool.tile(
                              [112, wid], cdt, tag="e", bufs=12,
                              name=f"et{s}_{cw[0]}_{h}",
                          )
                          nc.scalar.activation(e[:], ps_st[:], AF.Exp,
                                               scale=SCALE)
                          em = spool.tile(
                              [112, wid], cdt, tag="e", bufs=12,
                              name=f"em{s}_{cw[0]}_{h}",
                          )
                          nc.gpsimd.tensor_mul(em[:], e[:], mask4_t[:, 0:wid])
                          ems.append(em)
                      return {"ems": ems}

                  def tm_B(cw, hp, st):
                      wid = 112 * len(cw)
                      avs, rbs = [], []
                      for hi in range(2):
                          h = 6 + 2 * hp + hi
                          ps_av = ppool.tile(
                              [128, wid], F32, tag="ab", bufs=2,
                              name=f"ps_tav{s}_{cw[0]}_{h}",
                          )
                          for k, w in enumerate(cw):
                              nc.tensor.matmul(
                                  ps_av[:, 112 * k : 112 * k + 112],
                                  vt[w][:, 128 * (h - 6) : 128 * (h - 6) + 128],
                                  st["ems"][hi][:, 112 * k : 112 * k + 112],
                                  start=True,
                                  stop=True,
                              )
                          lt = spool.tile([64, wid], F32, tag="ln", bufs=4,
                                          name=f"lt{s}_{cw[0]}_{hp}_{hi}")
                          nc.scalar.activation(lt[:], ps_av[64:128, :], AF.Ln)
                          rb = spool.tile([64, wid], F32, tag="rb", bufs=4,
                                          name=f"rbt{s}_{cw[0]}_{hp}_{hi}")
                          nc.scalar.activation(rb[:], lt[:], AF.Exp,
                                               scale=-1.0)
                          avs.append(ps_av)
                          rbs.append(rb)
                      st["avs"] = avs
                      st["rbs"] = rbs

                  def tm_C(cw, hp, st):
                      wid = 112 * len(cw)
                      co = SB * hp + 112 * cw[0]
                      nc.vector.tensor_mul(
                          attnT[0:64, 3 * SB + co : 3 * SB + co + wid],
                          st["avs"][0][0:64, :], st["rbs"][0][:],
                      )
                      nc.vector.tensor_mul(
                          tmp_t[:, co : co + wid],
                          st["avs"][1][0:64, :], st["rbs"][1][:],
                      )

                  def shift_dma(dst0, tmp, c0, c1):
                      nc.sync.dma_start(
                          attnT[64:128, dst0 : dst0 + 3 * SB].rearrange(
                              "p (h c) -> p h c", c=SB)[:, :, c0:c1],
                          tmp.rearrange("p (h c) -> p h c", c=SB)[:, :, c0:c1],
                      )

                  # step list: 2 spatial : 1 temporal interleave
                  sp_steps = [("s", f, hp)
                              for f in range(FPSB) for hp in range(3)]
                  tm_steps = [("t", cw, hp)
                              for cw in ([0, 1, 2, 3], [4, 5, 6])
                              for hp in range(3)]
                  steps = []
                  si = ti = 0
                  while si < len(sp_steps) or ti < len(tm_steps):
                      for _ in range(2):
                          if si < len(sp_steps):
                              steps.append(sp_steps[si]); si += 1
                      if ti < len(tm_steps):
                          steps.append(tm_steps[ti]); ti += 1

                  hooks = {}
                  for n, (kind, a, hp) in enumerate(steps):
                      if kind == "s" and a == 1 and hp == 2:
                          hooks[n] = lambda: shift_dma(0, tmp_s, 0, 392)
                      elif kind == "s" and a == 3 and hp == 2:
                          hooks[n] = lambda: shift_dma(0, tmp_s, 392, 784)
                      elif kind == "t" and a[0] == 0 and hp == 2:
                          hooks[n] = lambda: shift_dma(3 * SB, tmp_t, 0, 448)
                      elif kind == "t" and a[0] == 4 and hp == 2:
                          hooks[n] = lambda: shift_dma(3 * SB, tmp_t, 448, 784)

                  state = {}
                  ticks = []

                  def tick(n):
                      if n < len(steps):
                          kind, a, hp = steps[n]
                          state[n] = sp_A(a, hp) if kind == "s" else tm_A(a, hp)
                      if n >= 1 and n - 1 < len(steps):
                          kind, a, hp = steps[n - 1]
                          (sp_B if kind == "s" else tm_B)(a, hp, state[n - 1])
                      if n >= 2:
                          m = n - 2
                          kind, a, hp = steps[m]
                          (sp_C if kind == "s" else tm_C)(a, hp, state.pop(m))
                          if m in hooks:
                              hooks[m]()

                  for n in range(len(steps) + 2):
                      ticks.append(lambda n=n: tick(n))
                  return ticks

              def out_units(s):
                  stt = sbst[s]
                  so = stt["so"]
                  if phase == "proj":
                      def dbg():
                          for ti in range(6):
                              nc.sync.dma_start(
                                  out_d.ap()[128 * ti : 128 * (ti + 1),
                                             so : so + SB],
                                  stt["qkvt"][ti][:],
                              )
                          for v in stt["vs"] + stt["vt"]:
                              nc.sync.dma_start(
                                  out_d.ap()[0 : v.shape[0], so : so + 768],
                                  v[:],
                              )
                      return [dbg]
                  attnT = stt["attnT"]
                  ot = wpool.tile([128, 6 * SB], cdt, tag="ot", bufs=1,
                                  name=f"ot_{s}")
                  units = []

                  def out_unit(j, ec):
                      ps = ppool.tile([128, 392], F32, tag="st", bufs=3,
                                      name=f"ps_o{s}_{ec}_{j}")
                      for dc in range(6):
                          nc.tensor.matmul(
                              ps[:],
                              mmcast(wp[dc][:, 128 * ec : 128 * (ec + 1)]),
                              mmcast(attnT[:, SB * dc + 392 * j
                                           : SB * dc + 392 * (j + 1)]),
                              start=(dc == 0),
                              stop=(dc == 5),
                          )
                      nc.vector.tensor_scalar_add(
                          ot[:, SB * ec + 392 * j : SB * ec + 392 * (j + 1)],
                          ps[:], bias_t[:, ec : ec + 1],
                      )

                  for j in range(2):
                      for ec in range(6):
                          units.append(lambda j=j, ec=ec: out_unit(j, ec))

                  def out_dma():
                      nc.sync.dma_start(
                          out_d.ap()[:, so : so + SB].rearrange(
                              "(c p) n -> p c n", p=128),
                          ot[:].rearrange("p (c n) -> p c n", n=SB),
                      )
                  units.append(out_dma)
                  return units

              # ---- phase loop: weave attention(p-1) with projections(p)
              # and out-proj(p-2) so the in-order PE queue always has dense
              # independent matmul work during attention's cross-engine
              # latency stalls.
              prefetch_x(0)
              for ph in range(NS + 2):
                  if ph + 1 < NS:
                      prefetch_x(ph + 1)
                  pu = proj_units(ph) if ph < NS else []
                  au = attn_emitters(ph - 1) if 0 <= ph - 1 < NS else []
                  ou = out_units(ph - 2) if 0 <= ph - 2 < NS else []
                  if not au:
                      for u in pu:
                          u()
                      for u in ou:
                          u()
                  else:
                      ap_acc = op_acc = 0.0
                      np_, no_ = len(pu), len(ou)
                      T = len(au)
                      pu = list(pu)
                      ou = list(ou)
                      for k in range(T):
                          au[k]()
                          ap_acc += np_ / T
                          while pu and ap_acc >= 1.0:
                              pu.pop(0)()
                              ap_acc -= 1.0
                          op_acc += no_ / T
                          while ou and op_acc >= 1.0:
                              ou.pop(0)()
                              op_acc -= 1.0
                      for u in pu:
                          u()
                      for u in ou:
                          u()
                  if ph - 2 >= 0:
                      sbst.pop(ph - 2, None)

    # Force every activation (Exp / Ln / Copy) onto the single
    # natural_log_exp_and_others table set: the default per-function pick
    # alternates exp_and_others <-> natural_log, inserting ~1.3us table
    # loads between almost every pair of activations. Restricting the
    # other sets' advertised coverage during this compile makes the
    # fixpoint pass emit one load with the correct act_func_set_id.
    import concourse.bacc as _bacc_mod

    _orig_tables = _bacc_mod.get_activation_tables

    def _single_set_tables(arch):
        t = dict(_orig_tables(arch))
        keep = t.get("natural_log_exp_and_others")
        if keep:
            t = {
                name: (fns if name == "natural_log_exp_and_others"
                       else fns - keep)
                for name, fns in t.items()
            }
        return t

    _bacc_mod.get_activation_tables = _single_set_tables
    try:
        nc.compile()
    finally:
        _bacc_mod.get_activation_tables = _orig_tables
    return nc


def _get_nc(compute: str):
    if compute not in _CACHE:
        _CACHE[compute] = _build(compute)
    return _CACHE[compute]


def _np_dtype(compute: str):
    if compute == "f32":
        return np.float32
    import ml_dtypes

    return ml_dtypes.bfloat16


def kernel(x, w_qkv, w_proj, b_proj):
    nc = _get_nc(COMPUTE)
    dt = _np_dtype(COMPUTE)

    x = np.asarray(x, dtype=np.float32).reshape(B, N, D)
    xT = np.ascontiguousarray(x.transpose(0, 2, 1)).astype(dt)  # (B, D, N)
    wqkvT = np.ascontiguousarray(np.asarray(w_qkv, np.float32).T).astype(dt)
    wprojT = np.ascontiguousarray(np.asarray(w_proj, np.float32).T).astype(dt)
    bias = np.asarray(b_proj, np.float32).reshape(D, 1)

    mask = np.zeros((112, 112), np.float32)
    for g in range(7):
        mask[16 * g : 16 * (g + 1), 16 * g : 16 * (g + 1)] = 1.0
    mask = mask.astype(dt)

    in_maps = [
        {"xt": xT[b], "wqkvT": wqkvT, "wprojT": wprojT, "bias": bias, "mask": mask}
        for b in range(B)
    ]
    res = run_bass_kernel_spmd(nc, in_maps, core_ids=list(range(B)))
    out = np.stack([np.asarray(r["outT"], dtype=np.float32).T for r in res.results])
    return np.ascontiguousarray(out.reshape(B, F, P, D)).astype(np.float32)


if __name__ == "__main__":
    rng = np.random.default_rng(0)
    x = rng.standard_normal((B, F, P, D), dtype=np.float32)
    w_qkv = rng.standard_normal((E3, D), dtype=np.float32) * D**-0.5
    w_proj = rng.standard_normal((D, D), dtype=np.float32) * D**-0.5
    b_proj = np.zeros(D, np.float32)
    out = kernel(x=x, w_qkv=w_qkv, w_proj=w_proj, b_proj=b_proj)
    print(out.shape, out.dtype)
